# revision 1
# baseline (speedup 1.0000x reference)
"""Trainium2 Bass kernel for MultiHeadLatentAttention (B=2, T=2048, C=2048, 16 heads).

Sharding over 8 NeuronCores: core c = (batch b = c//4, r = c%4).
 - Latent projections (x@wq_a, x@wkv_a) computed token-sharded (quarter r),
   in transposed layout (latent-dim on partitions), then AllGather-ed within
   each 4-core batch group.
 - Each core then handles head-group r (4 of 16 heads) for the full sequence:
   up-projections, RoPE+RMSNorm, block-causal attention, and a row-shard of
   the output projection.  Host sums the 4 partial outputs per batch.
All matmuls in bf16 with fp32 PSUM accumulation.
"""

from contextlib import ExitStack

import numpy as np
import ml_dtypes

import concourse.bass as bass
import concourse.tile as tile
import concourse.mybir as mybir
from concourse import bacc
from concourse.bass_utils import run_bass_kernel_spmd

BF16 = mybir.dt.bfloat16
F32 = mybir.dt.float32
NPBF16 = ml_dtypes.bfloat16

P = 128
B, T, C = 2, 2048, 2048
H, D = 16, 128
LORA = 1024
KV_PE = 256           # 2 chunks of 128
CONTENT = 768         # 6 chunks of 128
EPS = 1.1920929e-07
HG = 4                # heads per core
TQ = T // 4           # 512 tokens per quarter
NLB = LORA // P       # 8 latent row-blocks
NCC = C // P          # 16 contraction chunks of x
NTT = T // TQ         # 4 token 512-tiles
NKT = T // P          # 16 key tiles of 128
NQB = T // TQ         # 4 query blocks of 512
RG = [[0, 1, 2, 3], [4, 5, 6, 7]]

USE_AG = True

_NC_CACHE = {}


def build_nc(use_ag=USE_AG):
    nc = bacc.Bacc("TRN2", target_bir_lowering=False, debug=False, num_devices=8)

    xT = nc.dram_tensor("xT", [C, TQ if use_ag else T], BF16, kind="ExternalInput")
    wq_a = nc.dram_tensor("wq_a", [C, LORA], BF16, kind="ExternalInput")
    wkv_a = nc.dram_tensor("wkv_a", [C, LORA], BF16, kind="ExternalInput")
    wq_b = nc.dram_tensor("wq_b", [LORA, HG * D], BF16, kind="ExternalInput")
    wk_b = nc.dram_tensor("wk_b", [CONTENT, HG * D], BF16, kind="ExternalInput")
    wkpe_b = nc.dram_tensor("wkpe_b", [KV_PE, HG * D], BF16, kind="ExternalInput")
    wv_b = nc.dram_tensor("wv_b", [CONTENT, HG * D], BF16, kind="ExternalInput")
    wo = nc.dram_tensor("wo", [HG * D, C], BF16, kind="ExternalInput")
    cosT = nc.dram_tensor("cosT", [D // 2, T], F32, kind="ExternalInput")
    sinT = nc.dram_tensor("sinT", [D // 2, T], F32, kind="ExternalInput")
    masks = nc.dram_tensor("masks", [P, 4, TQ], BF16, kind="ExternalInput")
    outT = nc.dram_tensor("outT", [C, T], F32, kind="ExternalOutput")

    with tile.TileContext(nc) as tc, ExitStack() as ctx:
        dram = ctx.enter_context(tc.tile_pool(name="dram", bufs=1, space="DRAM"))
        psum = ctx.enter_context(tc.tile_pool(name="psum", bufs=8, space="PSUM"))
        consts = ctx.enter_context(tc.tile_pool(name="consts", bufs=1))
        persist = ctx.enter_context(tc.tile_pool(name="persist", bufs=1))
        tmp64 = ctx.enter_context(tc.tile_pool(name="tmp64", bufs=8))
        tmpk = ctx.enter_context(tc.tile_pool(name="tmpk", bufs=3))
        tmpsq = ctx.enter_context(tc.tile_pool(name="tmpsq", bufs=3))
        rows = ctx.enter_context(tc.tile_pool(name="rows", bufs=8))
        expool = ctx.enter_context(tc.tile_pool(name="expool", bufs=4))
        accpool = ctx.enter_context(tc.tile_pool(name="accpool", bufs=3))
        castpool = ctx.enter_context(tc.tile_pool(name="castpool", bufs=3))
        opool = ctx.enter_context(tc.tile_pool(name="opool", bufs=3))
        wop = ctx.enter_context(tc.tile_pool(name="wop", bufs=3))

        def ps_tile(name):
            return psum.tile([P, 512], F32, name=name, tag="ps")

        # ---- constants ----
        cos_sb = consts.tile([D // 2, T], F32, name="cos_sb")
        sin_sb = consts.tile([D // 2, T], F32, name="sin_sb")
        nc.sync.dma_start(out=cos_sb[:], in_=cosT[:])
        nc.sync.dma_start(out=sin_sb[:], in_=sinT[:])
        mask_sb = consts.tile([P, 4, TQ], BF16, name="mask_sb")
        nc.sync.dma_start(out=mask_sb[:], in_=masks[:])
        ones_red = consts.tile([P, 1], BF16, name="ones_red")
        nc.vector.memset(ones_red[:], 1.0)
        ones_bc = consts.tile([1, P], BF16, name="ones_bc")
        nc.vector.memset(ones_bc[:], 1.0)
        zeros128 = consts.tile([P, 1], F32, name="zeros128")
        nc.vector.memset(zeros128[:], 0.0)
        eps_k = consts.tile([1, 1], F32, name="eps_k")
        nc.vector.memset(eps_k[:], EPS)
        eps_q = consts.tile([1, 1], F32, name="eps_q")
        nc.vector.memset(eps_q[:], float(D) * EPS)

        # ---- persistent results of phase U ----
        kTn_sb = persist.tile([P, HG, T], BF16, name="kTn_sb")
        qTn_sb = persist.tile([P, HG, T], BF16, name="qTn_sb")
        v_sb = persist.tile([P, NKT, HG * D], BF16, name="v_sb")
        yTn_sb = persist.tile([P, HG, T], BF16, name="yTn_sb")

        # ---- phase L: latent projections (+ AllGather) ----
        if use_ag:
            cc_in_kv = dram.tile([LORA, TQ], BF16, name="cc_in_kv", tag="cc_in_kv")
            cc_out_kv = dram.tile([4 * LORA, TQ], BF16, name="cc_out_kv", tag="cc_out_kv")
            cc_in_q = dram.tile([LORA, TQ], BF16, name="cc_in_q", tag="cc_in_q")
            cc_out_q = dram.tile([4 * LORA, TQ], BF16, name="cc_out_q", tag="cc_out_q")

            with tc.tile_pool(name="xpool", bufs=16) as xpool, \
                 tc.tile_pool(name="wstream", bufs=3) as wsp, \
                 tc.tile_pool(name="latstage", bufs=2) as lsp:
                xsb = []
                for cc in range(NCC):
                    t = xpool.tile([P, TQ], BF16, name=f"xsb{cc}", tag="xsb")
                    nc.sync.dma_start(out=t[:], in_=xT[cc * P:(cc + 1) * P, :])
                    xsb.append(t)
                for wname, wh, ccin, ccout in [
                    ("kv", wkv_a, cc_in_kv, cc_out_kv),
                    ("q", wq_a, cc_in_q, cc_out_q),
                ]:
                    lat = lsp.tile([P, NLB, TQ], BF16, name=f"lat_{wname}", tag="lat")
                    pss = [ps_tile(f"lat_ps_{wname}{lb}") for lb in range(NLB)]
                    for cc in range(NCC):
                        wt = wsp.tile([P, LORA], BF16, name=f"wt_{wname}{cc}", tag="wt")
                        nc.sync.dma_start(out=wt[:], in_=wh[cc * P:(cc + 1) * P, :])
                        for lb in range(NLB):
                            nc.tensor.matmul(
                                pss[lb][:], wt[:, lb * P:(lb + 1) * P], xsb[cc][:],
                                start=(cc == 0), stop=(cc == NCC - 1))
                    for lb in range(NLB):
                        nc.scalar.copy(out=lat[:, lb, :], in_=pss[lb][:])
                    for lb in range(NLB):
                        nc.sync.dma_start(out=ccin[lb * P:(lb + 1) * P, :], in_=lat[:, lb, :])
                    nc.gpsimd.collective_compute(
                        "AllGather", mybir.AluOpType.bypass, replica_groups=RG,
                        ins=[ccin.opt()], outs=[ccout.opt()])

        # ---- phase U: up-projections per token-block ----
        def k_phase(h, tt, kvsb_t):
            kc_ps = ps_tile(f"kc_ps_{h}_{tt}")
            for j in range(CONTENT // P):
                nc.tensor.matmul(kc_ps[:], wkb_sb[:, j, h * D:(h + 1) * D],
                                 kvsb_t[:, 2 + j, :], start=(j == 0), stop=(j == 5))
            kpe_ps = ps_tile(f"kpe_ps_{h}_{tt}")
            for j in range(KV_PE // P):
                nc.tensor.matmul(kpe_ps[:], wkpe_sb[:, j, h * D:(h + 1) * D],
                                 kvsb_t[:, j, :], start=(j == 0), stop=(j == 1))
            cos_t = cos_sb[:, tt * TQ:(tt + 1) * TQ]
            sin_t = sin_sb[:, tt * TQ:(tt + 1) * TQ]
            hd = D // 2
            m1 = tmp64.tile([hd, TQ], F32, name=f"m1_{h}_{tt}", tag="m64")
            m2 = tmp64.tile([hd, TQ], F32, name=f"m2_{h}_{tt}", tag="m64")
            m3 = tmp64.tile([hd, TQ], F32, name=f"m3_{h}_{tt}", tag="m64")
            m4 = tmp64.tile([hd, TQ], F32, name=f"m4_{h}_{tt}", tag="m64")
            nc.vector.tensor_mul(m1[:], kpe_ps[0:hd, :], cos_t)
            nc.vector.tensor_mul(m2[:], kpe_ps[hd:D, :], sin_t)
            nc.vector.tensor_mul(m3[:], kpe_ps[0:hd, :], sin_t)
            nc.vector.tensor_mul(m4[:], kpe_ps[hd:D, :], cos_t)
            k_un = tmpk.tile([P, TQ], F32, name=f"k_un_{h}_{tt}", tag="k_un")
            nc.vector.tensor_add(m1[:], m1[:], m2[:])
            nc.vector.tensor_add(k_un[0:hd, :], m1[:], kc_ps[0:hd, :])
            nc.vector.tensor_sub(m4[:], m4[:], m3[:])
            nc.vector.tensor_add(k_un[hd:D, :], m4[:], kc_ps[hd:D, :])
            # rmsnorm scale: r = 1/sqrt(mean(k^2)+eps), via ones-matmul partition sum
            sq = tmpsq.tile([P, TQ], BF16, name=f"ksq_{h}_{tt}", tag="sq")
            nc.vector.tensor_mul(sq[:], k_un[:], k_un[:])
            ss_ps = ps_tile(f"kss_ps_{h}_{tt}")
            nc.tensor.matmul(ss_ps[0:1, :], ones_red[:], sq[:], start=True, stop=True)
            sroot = rows.tile([1, TQ], F32, name=f"ksroot_{h}_{tt}", tag="row_f")
            nc.scalar.activation(sroot[:], ss_ps[0:1, :],
                                 mybir.ActivationFunctionType.Sqrt,
                                 bias=eps_k[:], scale=1.0 / D)
            rrow = rows.tile([1, TQ], F32, name=f"krrow_{h}_{tt}", tag="row_f")
            nc.vector.reciprocal(rrow[:], sroot[:])
            rbf = rows.tile([1, TQ], BF16, name=f"krbf_{h}_{tt}", tag="row_b")
            nc.vector.tensor_copy(out=rbf[:], in_=rrow[:])
            bc_ps = ps_tile(f"kbc_ps_{h}_{tt}")
            nc.tensor.matmul(bc_ps[:], ones_bc[:], rbf[:], start=True, stop=True)
            nc.vector.tensor_mul(kTn_sb[:, h, tt * TQ:(tt + 1) * TQ], k_un[:], bc_ps[:])

        def q_phase(h, tt, qlsb_t):
            q_ps = ps_tile(f"q_ps_{h}_{tt}")
            for j in range(NLB):
                nc.tensor.matmul(q_ps[:], wqb_sb[:, j, h * D:(h + 1) * D],
                                 qlsb_t[:, j, :], start=(j == 0), stop=(j == NLB - 1))
            sq = tmpsq.tile([P, TQ], BF16, name=f"qsq_{h}_{tt}", tag="sq")
            nc.scalar.activation(sq[:], q_ps[:], mybir.ActivationFunctionType.Square,
                                 bias=zeros128[:], scale=1.0)
            ss_ps = ps_tile(f"qss_ps_{h}_{tt}")
            nc.tensor.matmul(ss_ps[0:1, :], ones_red[:], sq[:], start=True, stop=True)
            # sqrt(sumsq + D*eps) = sqrt(D) * sqrt(mean+eps); recip gives r_q/sqrt(D)
            sroot = rows.tile([1, TQ], F32, name=f"qsroot_{h}_{tt}", tag="row_f")
            nc.scalar.activation(sroot[:], ss_ps[0:1, :],
                                 mybir.ActivationFunctionType.Sqrt,
                                 bias=eps_q[:], scale=1.0)
            rrow = rows.tile([1, TQ], F32, name=f"qrrow_{h}_{tt}", tag="row_f")
            nc.vector.reciprocal(rrow[:], sroot[:])
            rbf = rows.tile([1, TQ], BF16, name=f"qrbf_{h}_{tt}", tag="row_b")
            nc.vector.tensor_copy(out=rbf[:], in_=rrow[:])
            bc_ps = ps_tile(f"qbc_ps_{h}_{tt}")
            nc.tensor.matmul(bc_ps[:], ones_bc[:], rbf[:], start=True, stop=True)
            qc = castpool.tile([P, TQ], BF16, name=f"qc_{h}_{tt}", tag="cast")
            nc.scalar.copy(out=qc[:], in_=q_ps[:])
            nc.vector.tensor_mul(qTn_sb[:, h, tt * TQ:(tt + 1) * TQ], qc[:], bc_ps[:])

        def v_phase(tt, t4, kvsb_t):
            v_ps = ps_tile(f"v_ps_{tt}_{t4}")
            for j in range(CONTENT // P):
                nc.tensor.matmul(v_ps[:], kvsb_t[:, 2 + j, t4 * P:(t4 + 1) * P],
                                 wv_sb[:, j, :], start=(j == 0), stop=(j == 5))
            nc.scalar.copy(out=v_sb[:, tt * 4 + t4, :], in_=v_ps[:])

        with tc.tile_pool(name="wu", bufs=1) as wu, \
             tc.tile_pool(name="kvpool", bufs=2) as kvpool, \
             tc.tile_pool(name="qlpool", bufs=2) as qlpool, \
             tc.tile_pool(name="xpool2", bufs=16) as xpool2, \
             tc.tile_pool(name="wstream2", bufs=3) as wsp2:
            wkb_sb = wu.tile([P, CONTENT // P, HG * D], BF16, name="wkb_sb")
            nc.sync.dma_start(out=wkb_sb[:], in_=wk_b.rearrange("(j p) n -> p j n", p=P))
            wkpe_sb = wu.tile([P, KV_PE // P, HG * D], BF16, name="wkpe_sb")
            nc.sync.dma_start(out=wkpe_sb[:], in_=wkpe_b.rearrange("(j p) n -> p j n", p=P))
            wv_sb = wu.tile([P, CONTENT // P, HG * D], BF16, name="wv_sb")
            nc.sync.dma_start(out=wv_sb[:], in_=wv_b.rearrange("(j p) n -> p j n", p=P))
            wqb_sb = wu.tile([P, NLB, HG * D], BF16, name="wqb_sb")
            nc.sync.dma_start(out=wqb_sb[:], in_=wq_b.rearrange("(j p) n -> p j n", p=P))

            for tt in range(NTT):
                if use_ag:
                    kvsb_t = kvpool.tile([P, NLB, TQ], BF16, name=f"kvsb{tt}", tag="kvsb")
                    nc.sync.dma_start(
                        out=kvsb_t[:],
                        in_=cc_out_kv[LORA * tt:LORA * (tt + 1), :].rearrange(
                            "(c p) t -> p c t", p=P))
                    qlsb_t = qlpool.tile([P, NLB, TQ], BF16, name=f"qlsb{tt}", tag="qlsb")
                    nc.sync.dma_start(
                        out=qlsb_t[:],
                        in_=cc_out_q[LORA * tt:LORA * (tt + 1), :].rearrange(
                            "(c p) t -> p c t", p=P))
                else:
                    xsb2 = []
                    for cc in range(NCC):
                        t = xpool2.tile([P, TQ], BF16, name=f"x2_{tt}_{cc}", tag="xsb2")
                        nc.sync.dma_start(out=t[:], in_=xT[cc * P:(cc + 1) * P,
                                                          tt * TQ:(tt + 1) * TQ])
                        xsb2.append(t)
                    kvsb_t = kvpool.tile([P, NLB, TQ], BF16, name=f"kvsb{tt}", tag="kvsb")
                    qlsb_t = qlpool.tile([P, NLB, TQ], BF16, name=f"qlsb{tt}", tag="qlsb")
                    for wname, wh, dst in [("kv", wkv_a, kvsb_t), ("q", wq_a, qlsb_t)]:
                        pss = [ps_tile(f"lat_ps_{wname}_{tt}_{lb}") for lb in range(NLB)]
                        for cc in range(NCC):
                            wt = wsp2.tile([P, LORA], BF16, name=f"w2_{wname}_{tt}_{cc}", tag="wt2")
                            nc.sync.dma_start(out=wt[:], in_=wh[cc * P:(cc + 1) * P, :])
                            for lb in range(NLB):
                                nc.tensor.matmul(
                                    pss[lb][:], wt[:, lb * P:(lb + 1) * P], xsb2[cc][:],
                                    start=(cc == 0), stop=(cc == NCC - 1))
                        for lb in range(NLB):
                            nc.scalar.copy(out=dst[:, lb, :], in_=pss[lb][:])

                for h in range(HG):
                    k_phase(h, tt, kvsb_t)
                for t4 in range(4):
                    v_phase(tt, t4, kvsb_t)
                for h in range(HG):
                    q_phase(h, tt, qlsb_t)

        # ---- phase A: block-causal attention ----
        for h in range(HG):
            for qb in range(NQB):
                nkt = 4 * (qb + 1)
                yt_ps = ps_tile(f"yt_ps_{h}_{qb}")
                acc = accpool.tile([P, TQ], BF16, name=f"acc_{h}_{qb}", tag="acc")
                for kt in range(nkt):
                    sc_ps = ps_tile(f"sc_ps_{h}_{qb}_{kt}")
                    nc.tensor.matmul(sc_ps[:], kTn_sb[:, h, kt * P:(kt + 1) * P],
                                     qTn_sb[:, h, qb * TQ:(qb + 1) * TQ],
                                     start=True, stop=True)
                    ex = expool.tile([P, TQ], BF16, name=f"ex_{h}_{qb}_{kt}", tag="ex")
                    nc.scalar.activation(ex[:], sc_ps[:],
                                         mybir.ActivationFunctionType.Exp,
                                         bias=zeros128[:], scale=1.0)
                    if kt >= 4 * qb:
                        nc.vector.tensor_mul(ex[:], ex[:], mask_sb[:, kt - 4 * qb, :])
                    if kt == 0:
                        nc.vector.tensor_copy(out=acc[:], in_=ex[:])
                    else:
                        nc.vector.tensor_add(acc[:], acc[:], ex[:])
                    nc.tensor.matmul(yt_ps[:], v_sb[:, kt, h * D:(h + 1) * D], ex[:],
                                     start=(kt == 0), stop=(kt == nkt - 1))
                den_ps = ps_tile(f"den_ps_{h}_{qb}")
                nc.tensor.matmul(den_ps[0:1, :], ones_red[:], acc[:], start=True, stop=True)
                rden = rows.tile([1, TQ], F32, name=f"rden_{h}_{qb}", tag="row_f")
                nc.vector.reciprocal(rden[:], den_ps[0:1, :])
                rdbf = rows.tile([1, TQ], BF16, name=f"rdbf_{h}_{qb}", tag="row_b")
                nc.vector.tensor_copy(out=rdbf[:], in_=rden[:])
                bc_ps = ps_tile(f"abc_ps_{h}_{qb}")
                nc.tensor.matmul(bc_ps[:], ones_bc[:], rdbf[:], start=True, stop=True)
                yc = castpool.tile([P, TQ], BF16, name=f"yc_{h}_{qb}", tag="cast")
                nc.scalar.copy(out=yc[:], in_=yt_ps[:])
                nc.vector.tensor_mul(yTn_sb[:, h, qb * TQ:(qb + 1) * TQ], yc[:], bc_ps[:])

        # ---- phase O: output projection (row-shard of wo) ----
        for ct in range(C // P):
            wo_t = wop.tile([P, HG, P], BF16, name=f"wo_t{ct}", tag="wo_t")
            nc.sync.dma_start(out=wo_t[:],
                              in_=wo[:, ct * P:(ct + 1) * P].rearrange("(h p) c -> p h c", p=P))
            for tt in range(NTT):
                o_ps = ps_tile(f"o_ps_{ct}_{tt}")
                for h in range(HG):
                    nc.tensor.matmul(o_ps[:], wo_t[:, h, :],
                                     yTn_sb[:, h, tt * TQ:(tt + 1) * TQ],
                                     start=(h == 0), stop=(h == HG - 1))
                o_sb = opool.tile([P, TQ], F32, name=f"o_sb_{ct}_{tt}", tag="o_sb")
                nc.vector.tensor_copy(out=o_sb[:], in_=o_ps[:])
                nc.sync.dma_start(out=outT[ct * P:(ct + 1) * P, tt * TQ:(tt + 1) * TQ],
                                  in_=o_sb[:])

    nc.compile()
    return nc


def _get_nc(use_ag=USE_AG):
    if use_ag not in _NC_CACHE:
        _NC_CACHE[use_ag] = build_nc(use_ag)
    return _NC_CACHE[use_ag]


def _prepare_in_maps(x, cos, sin, wq_a, wq_b, wkv_a, wk_b, wkpe_b, wv_b, wo, use_ag=USE_AG):
    def bf(a):
        return np.ascontiguousarray(a).astype(NPBF16)

    cosT = np.ascontiguousarray(np.asarray(cos, np.float32)[0, :, 0, :].T)
    sinT = np.ascontiguousarray(np.asarray(sin, np.float32)[0, :, 0, :].T)
    p_idx = np.arange(P)[:, None, None]
    j_idx = np.arange(4)[None, :, None]
    t_idx = np.arange(TQ)[None, None, :]
    masks = ((P * j_idx + p_idx) <= t_idx).astype(NPBF16)

    wq_a_b, wkv_a_b = bf(wq_a), bf(wkv_a)
    wq_b_b, wk_b_b = bf(wq_b), bf(wk_b)
    wkpe_b_b, wv_b_b, wo_b = bf(wkpe_b), bf(wv_b), bf(wo)
    x = np.asarray(x, np.float32)

    in_maps = []
    for c in range(8):
        b, r = c // 4, c % 4
        if use_ag:
            xT_c = bf(x[b, r * TQ:(r + 1) * TQ, :].T)
        else:
            xT_c = bf(x[b].T)
        hgs = slice(r * HG * D, (r + 1) * HG * D)
        in_maps.append({
            "xT": xT_c,
            "wq_a": wq_a_b,
            "wkv_a": wkv_a_b,
            "wq_b": np.ascontiguousarray(wq_b_b[:, hgs]),
            "wk_b": np.ascontiguousarray(wk_b_b[:, hgs]),
            "wkpe_b": np.ascontiguousarray(wkpe_b_b[:, hgs]),
            "wv_b": np.ascontiguousarray(wv_b_b[:, hgs]),
            "wo": np.ascontiguousarray(wo_b[hgs, :]),
            "cosT": cosT,
            "sinT": sinT,
            "masks": masks,
        })
    return in_maps


def _assemble(results):
    out = np.empty((B, T, C), np.float32)
    for b in range(B):
        acc = results[4 * b]["outT"].astype(np.float32).copy()
        for r in range(1, 4):
            acc += results[4 * b + r]["outT"]
        out[b] = acc.T
    return out


def _run(inputs, use_ag=USE_AG, trace=False):
    nc = _get_nc(use_ag)
    in_maps = _prepare_in_maps(use_ag=use_ag, **inputs)
    res = run_bass_kernel_spmd(nc, in_maps, core_ids=list(range(8)), trace=trace)
    return _assemble(res.results), res


def kernel(**inputs):
    out, _ = _run(inputs)
    return out


# revision 6
# speedup vs baseline: 1.5174x; 1.5174x over previous
"""Trainium2 Bass kernel for MultiHeadLatentAttention (B=2, T=2048, C=2048, 16 heads).

Sharding over 8 NeuronCores: core c = (batch b = c//4, r = c%4).
 - Latent projections (x@wq_a, x@wkv_a) computed token-sharded (quarter r),
   in transposed layout (latent-dim on partitions), then AllGather-ed within
   each 4-core batch group.
 - Each core then handles head-group r (4 of 16 heads) for the full sequence:
   up-projections, RoPE+RMSNorm, block-causal attention, and a row-shard of
   the output projection.  Host sums the 4 partial outputs per batch.
All matmuls in bf16 with fp32 PSUM accumulation.  RMS/softmax denominators
are computed via ones-matmul partition reductions batched into 32-aligned
PSUM rows so one DVE reciprocal serves four rows; per-row broadcasts are
selector-matmuls.  Norm/softmax tails are software-pipelined one block behind
the matmul stream so the PE never waits on the vector-engine chains.
"""

from contextlib import ExitStack

import numpy as np
import ml_dtypes

import concourse.bass as bass
import concourse.tile as tile
import concourse.mybir as mybir
from concourse import bacc
from concourse.bass_utils import run_bass_kernel_spmd

BF16 = mybir.dt.bfloat16
F32 = mybir.dt.float32
NPBF16 = ml_dtypes.bfloat16
AF = mybir.ActivationFunctionType

P = 128
B, T, C = 2, 2048, 2048
H, D = 16, 128
LORA = 1024
KV_PE = 256           # 2 chunks of 128
CONTENT = 768         # 6 chunks of 128
EPS = 1.1920929e-07
HG = 4                # heads per core
TQ = 512              # tokens per quarter / query block
NLB = LORA // P       # 8 latent row-blocks
NCC = C // P          # 16 contraction chunks of x
NTT = T // TQ         # 4 token 512-tiles
NKT = T // P          # 16 key tiles of 128
NQB = T // TQ         # 4 query blocks of 512
RG = [[0, 1, 2, 3], [4, 5, 6, 7]]

USE_AG = True

_NC_CACHE = {}


def build_nc(use_ag=USE_AG):
    nc = bacc.Bacc("TRN2", target_bir_lowering=False, debug=False, num_devices=8)

    xT = nc.dram_tensor("xT", [C, TQ if use_ag else T], BF16, kind="ExternalInput")
    wq_a = nc.dram_tensor("wq_a", [C, LORA], BF16, kind="ExternalInput")
    wkv_a = nc.dram_tensor("wkv_a", [C, LORA], BF16, kind="ExternalInput")
    wq_b = nc.dram_tensor("wq_b", [LORA, HG * D], BF16, kind="ExternalInput")
    wk_b = nc.dram_tensor("wk_b", [CONTENT, HG * D], BF16, kind="ExternalInput")
    wkpe_b = nc.dram_tensor("wkpe_b", [KV_PE, HG * D], BF16, kind="ExternalInput")
    wv_b = nc.dram_tensor("wv_b", [CONTENT, HG * D], BF16, kind="ExternalInput")
    wo = nc.dram_tensor("wo", [HG * D, C], BF16, kind="ExternalInput")
    cosT = nc.dram_tensor("cosT", [D // 2, T], BF16, kind="ExternalInput")
    sinT = nc.dram_tensor("sinT", [D // 2, T], BF16, kind="ExternalInput")
    tri = nc.dram_tensor("tri", [P, P], BF16, kind="ExternalInput")
    outT = nc.dram_tensor("outT", [C, T], F32, kind="ExternalOutput")

    with tile.TileContext(nc) as tc, ExitStack() as ctx:
        dram = ctx.enter_context(tc.tile_pool(name="dram", bufs=1, space="DRAM"))
        psum = ctx.enter_context(tc.tile_pool(name="psum", bufs=8, space="PSUM"))
        consts = ctx.enter_context(tc.tile_pool(name="consts", bufs=1))
        persist = ctx.enter_context(tc.tile_pool(name="persist", bufs=1))
        tmpk = ctx.enter_context(tc.tile_pool(name="tmpk", bufs=8))
        tmp64 = ctx.enter_context(tc.tile_pool(name="tmp64", bufs=8))
        kcbf = ctx.enter_context(tc.tile_pool(name="kcbf", bufs=4))
        tmpsq = ctx.enter_context(tc.tile_pool(name="tmpsq", bufs=4))
        normf = ctx.enter_context(tc.tile_pool(name="normf", bufs=3))
        normb = ctx.enter_context(tc.tile_pool(name="normb", bufs=3))
        expool = ctx.enter_context(tc.tile_pool(name="expool", bufs=6))
        accpool = ctx.enter_context(tc.tile_pool(name="accpool", bufs=4))
        castpool = ctx.enter_context(tc.tile_pool(name="castpool", bufs=10))
        opool = ctx.enter_context(tc.tile_pool(name="opool", bufs=6))
        wop = ctx.enter_context(tc.tile_pool(name="wop", bufs=3))

        def ps_tile(name):
            return psum.tile([P, 512], F32, name=name, tag="ps")

        def row_mm(out_tile, h, lhsT, rhs):
            # ones-matmul partition reduction into 32-aligned row 32*h.
            # Each row-MM is its own complete accumulation group: the rows are
            # disjoint so Tile sees no deps between them and may reorder; a
            # shared group would then accumulate onto stale bank bits.
            tp = (0, 32 * h) if h == 3 else None
            nc.tensor.matmul(out_tile[32 * h:32 * h + 1, :], lhsT, rhs,
                             start=True, stop=True, tile_position=tp)

        # ---- constants ----
        cos_sb = consts.tile([D // 2, T], BF16, name="cos_sb")
        sin_sb = consts.tile([D // 2, T], BF16, name="sin_sb")
        nc.sync.dma_start(out=cos_sb[:], in_=cosT[:])
        nc.sync.dma_start(out=sin_sb[:], in_=sinT[:])
        tri_sb = consts.tile([P, P], BF16, name="tri_sb")
        nc.sync.dma_start(out=tri_sb[:], in_=tri[:])
        ones_red = consts.tile([P, 1], BF16, name="ones_red")
        nc.vector.memset(ones_red[:], 1.0)
        zeros128 = consts.tile([P, 1], F32, name="zeros128")
        nc.vector.memset(zeros128[:], 0.0)
        eps_k128 = consts.tile([P, 1], F32, name="eps_k128")
        nc.vector.memset(eps_k128[:], EPS)
        eps_q128 = consts.tile([P, 1], F32, name="eps_q128")
        nc.vector.memset(eps_q128[:], float(D) * EPS)
        sels = []
        for j in range(4):
            s = consts.tile([P, P], BF16, name=f"sel{j}")
            nc.vector.memset(s[:], 0.0)
            nc.vector.memset(s[32 * j:32 * j + 1, :], 1.0)
            sels.append(s)

        # ---- persistent results of phase U ----
        kTn_sb = persist.tile([P, HG, T], BF16, name="kTn_sb")
        qTn_sb = persist.tile([P, HG, T], BF16, name="qTn_sb")
        v_sb = persist.tile([P, NKT, HG * D], BF16, name="v_sb")
        yTn_sb = persist.tile([P, HG, T], BF16, name="yTn_sb")

        # ---- phase L: latent projections (+ AllGather) ----
        if use_ag:
            cc_in_kv = dram.tile([LORA, TQ], BF16, name="cc_in_kv", tag="cc_in_kv")
            cc_out_kv = dram.tile([4 * LORA, TQ], BF16, name="cc_out_kv", tag="cc_out_kv")
            cc_in_q = dram.tile([LORA, TQ], BF16, name="cc_in_q", tag="cc_in_q")
            cc_out_q = dram.tile([4 * LORA, TQ], BF16, name="cc_out_q", tag="cc_out_q")

            with tc.tile_pool(name="xpool", bufs=16) as xpool, \
                 tc.tile_pool(name="wstream", bufs=3) as wsp, \
                 tc.tile_pool(name="latstage", bufs=2) as lsp:
                xsb = []
                for cc in range(NCC):
                    t = xpool.tile([P, TQ], BF16, name=f"xsb{cc}", tag="xsb")
                    nc.sync.dma_start(out=t[:], in_=xT[cc * P:(cc + 1) * P, :])
                    xsb.append(t)
                for wname, wh, ccin, ccout in [
                    ("kv", wkv_a, cc_in_kv, cc_out_kv),
                    ("q", wq_a, cc_in_q, cc_out_q),
                ]:
                    lat = lsp.tile([P, NLB, TQ], BF16, name=f"lat_{wname}", tag="lat")
                    pss = [ps_tile(f"lat_ps_{wname}{lb}") for lb in range(NLB)]
                    for cc in range(NCC):
                        wt = wsp.tile([P, LORA], BF16, name=f"wt_{wname}{cc}", tag="wt")
                        nc.sync.dma_start(out=wt[:], in_=wh[cc * P:(cc + 1) * P, :])
                        for lb in range(NLB):
                            nc.tensor.matmul(
                                pss[lb][:], wt[:, lb * P:(lb + 1) * P], xsb[cc][:],
                                start=(cc == 0), stop=(cc == NCC - 1))
                    for lb in range(NLB):
                        nc.scalar.copy(out=lat[:, lb, :], in_=pss[lb][:])
                    for lb in range(NLB):
                        nc.sync.dma_start(out=ccin[lb * P:(lb + 1) * P, :], in_=lat[:, lb, :])
                    nc.gpsimd.collective_compute(
                        "AllGather", mybir.AluOpType.bypass, replica_groups=RG,
                        ins=[ccin.opt()], outs=[ccout.opt()])

        # ---- phase U: up-projections + RoPE + RMS-norm scales ----
        def k_head(h, tt, kvsb_t, ss_k, kuns):
            kc_ps = ps_tile(f"kc_ps_{h}_{tt}")
            for j in range(CONTENT // P):
                nc.tensor.matmul(kc_ps[:], wkb_sb[:, j, h * D:(h + 1) * D],
                                 kvsb_t[:, 2 + j, :], start=(j == 0), stop=(j == 5))
            kpe_ps = ps_tile(f"kpe_ps_{h}_{tt}")
            for j in range(KV_PE // P):
                nc.tensor.matmul(kpe_ps[:], wkpe_sb[:, j, h * D:(h + 1) * D],
                                 kvsb_t[:, j, :], start=(j == 0), stop=(j == 1))
            cos_t = cos_sb[:, tt * TQ:(tt + 1) * TQ]
            sin_t = sin_sb[:, tt * TQ:(tt + 1) * TQ]
            hd = D // 2
            m1 = tmp64.tile([hd, TQ], BF16, name=f"m1_{h}_{tt}", tag="m64")
            m2 = tmp64.tile([hd, TQ], BF16, name=f"m2_{h}_{tt}", tag="m64")
            m3 = tmp64.tile([hd, TQ], BF16, name=f"m3_{h}_{tt}", tag="m64")
            m4 = tmp64.tile([hd, TQ], BF16, name=f"m4_{h}_{tt}", tag="m64")
            nc.vector.tensor_mul(m1[:], kpe_ps[0:hd, :], cos_t)
            nc.vector.tensor_mul(m2[:], kpe_ps[hd:D, :], sin_t)
            nc.vector.tensor_mul(m3[:], kpe_ps[0:hd, :], sin_t)
            nc.vector.tensor_mul(m4[:], kpe_ps[hd:D, :], cos_t)
            k_un = tmpk.tile([P, TQ], BF16, name=f"k_un_{h}_{tt}", tag="k_un")
            nc.vector.tensor_add(m1[:], m1[:], m2[:])
            nc.vector.tensor_add(k_un[0:hd, :], m1[:], kc_ps[0:hd, :])
            nc.vector.tensor_sub(m4[:], m4[:], m3[:])
            nc.vector.tensor_add(k_un[hd:D, :], m4[:], kc_ps[hd:D, :])
            kuns.append(k_un)
            sq = tmpsq.tile([P, TQ], BF16, name=f"ksq_{h}_{tt}", tag="sq")
            nc.vector.tensor_mul(sq[:], k_un[:], k_un[:])
            row_mm(ss_k, h, ones_red[:], sq[:])

        def q_head(h, tt, qlsb_t, ss_q, qcs):
            q_ps = ps_tile(f"q_ps_{h}_{tt}")
            for j in range(NLB):
                nc.tensor.matmul(q_ps[:], wqb_sb[:, j, h * D:(h + 1) * D],
                                 qlsb_t[:, j, :], start=(j == 0), stop=(j == NLB - 1))
            qc = castpool.tile([P, TQ], BF16, name=f"qc_{h}_{tt}", tag="cast")
            nc.scalar.copy(out=qc[:], in_=q_ps[:])
            qcs.append(qc)
            sq = tmpsq.tile([P, TQ], BF16, name=f"qsq_{h}_{tt}", tag="sq")
            nc.scalar.activation(sq[:], q_ps[:], AF.Square, bias=zeros128[:], scale=1.0)
            row_mm(ss_q, h, ones_red[:], sq[:])

        def v_block(tt, t4, kvsb_t):
            v_ps = ps_tile(f"v_ps_{tt}_{t4}")
            for j in range(CONTENT // P):
                nc.tensor.matmul(v_ps[:], kvsb_t[:, 2 + j, t4 * P:(t4 + 1) * P],
                                 wv_sb[:, j, :], start=(j == 0), stop=(j == 5))
            nc.scalar.copy(out=v_sb[:, tt * 4 + t4, :], in_=v_ps[:])

        def u_tail(tt, ss_k, ss_q, kuns, qcs):
            # k scale: 1/sqrt(sumsq/D + eps); q scale: 1/sqrt(sumsq + D*eps)
            for which, ss, scale, bias_t, srcs, dst in (
                ("k", ss_k, 1.0 / D, eps_k128, kuns, kTn_sb),
                ("q", ss_q, 1.0, eps_q128, qcs, qTn_sb),
            ):
                sroot = normf.tile([P, TQ], F32, name=f"sroot_{which}_{tt}", tag="nf")
                nc.scalar.activation(sroot[:], ss[:], AF.Sqrt, bias=bias_t[:], scale=scale)
                rinv = normf.tile([P, TQ], F32, name=f"rinv_{which}_{tt}", tag="nf")
                nc.vector.reciprocal(rinv[:], sroot[:])
                rbf = normb.tile([P, TQ], BF16, name=f"rbf_{which}_{tt}", tag="nb")
                nc.vector.tensor_copy(out=rbf[:], in_=rinv[:])
                for h in range(HG):
                    bc = ps_tile(f"bc_{which}_{h}_{tt}")
                    nc.tensor.matmul(bc[:], sels[h][:], rbf[:], start=True, stop=True)
                    nc.vector.tensor_mul(dst[:, h, tt * TQ:(tt + 1) * TQ],
                                         srcs[h][:], bc[:])

        with tc.tile_pool(name="wu", bufs=1) as wu, \
             tc.tile_pool(name="kvpool", bufs=2) as kvpool, \
             tc.tile_pool(name="qlpool", bufs=2) as qlpool, \
             tc.tile_pool(name="xpool2", bufs=16) as xpool2, \
             tc.tile_pool(name="wstream2", bufs=3) as wsp2:
            wkb_sb = wu.tile([P, CONTENT // P, HG * D], BF16, name="wkb_sb")
            nc.sync.dma_start(out=wkb_sb[:], in_=wk_b.rearrange("(j p) n -> p j n", p=P))
            wkpe_sb = wu.tile([P, KV_PE // P, HG * D], BF16, name="wkpe_sb")
            nc.sync.dma_start(out=wkpe_sb[:], in_=wkpe_b.rearrange("(j p) n -> p j n", p=P))
            wv_sb = wu.tile([P, CONTENT // P, HG * D], BF16, name="wv_sb")
            nc.sync.dma_start(out=wv_sb[:], in_=wv_b.rearrange("(j p) n -> p j n", p=P))
            wqb_sb = wu.tile([P, NLB, HG * D], BF16, name="wqb_sb")
            nc.sync.dma_start(out=wqb_sb[:], in_=wq_b.rearrange("(j p) n -> p j n", p=P))

            pending_u = []
            for tt in range(NTT):
                if use_ag:
                    kvsb_t = kvpool.tile([P, NLB, TQ], BF16, name=f"kvsb{tt}", tag="kvsb")
                    nc.sync.dma_start(
                        out=kvsb_t[:],
                        in_=cc_out_kv[LORA * tt:LORA * (tt + 1), :].rearrange(
                            "(c p) t -> p c t", p=P))
                    qlsb_t = qlpool.tile([P, NLB, TQ], BF16, name=f"qlsb{tt}", tag="qlsb")
                    nc.sync.dma_start(
                        out=qlsb_t[:],
                        in_=cc_out_q[LORA * tt:LORA * (tt + 1), :].rearrange(
                            "(c p) t -> p c t", p=P))
                else:
                    xsb2 = []
                    for cc in range(NCC):
                        t = xpool2.tile([P, TQ], BF16, name=f"x2_{tt}_{cc}", tag="xsb2")
                        nc.sync.dma_start(out=t[:], in_=xT[cc * P:(cc + 1) * P,
                                                          tt * TQ:(tt + 1) * TQ])
                        xsb2.append(t)
                    kvsb_t = kvpool.tile([P, NLB, TQ], BF16, name=f"kvsb{tt}", tag="kvsb")
                    qlsb_t = qlpool.tile([P, NLB, TQ], BF16, name=f"qlsb{tt}", tag="qlsb")
                    for wname, wh, dst in [("kv", wkv_a, kvsb_t), ("q", wq_a, qlsb_t)]:
                        pss = [ps_tile(f"lat_ps_{wname}_{tt}_{lb}") for lb in range(NLB)]
                        for cc in range(NCC):
                            wt = wsp2.tile([P, LORA], BF16, name=f"w2_{wname}_{tt}_{cc}", tag="wt2")
                            nc.sync.dma_start(out=wt[:], in_=wh[cc * P:(cc + 1) * P, :])
                            for lb in range(NLB):
                                nc.tensor.matmul(
                                    pss[lb][:], wt[:, lb * P:(lb + 1) * P], xsb2[cc][:],
                                    start=(cc == 0), stop=(cc == NCC - 1))
                        for lb in range(NLB):
                            nc.scalar.copy(out=dst[:, lb, :], in_=pss[lb][:])

                ss_k = ps_tile(f"ss_k_{tt}")
                nc.vector.memset(ss_k[:], 0.0)
                ss_q = ps_tile(f"ss_q_{tt}")
                nc.vector.memset(ss_q[:], 0.0)
                kuns, qcs = [], []
                for h in range(HG):
                    k_head(h, tt, kvsb_t, ss_k, kuns)
                if pending_u:
                    u_tail(*pending_u.pop(0))
                for t4 in range(4):
                    v_block(tt, t4, kvsb_t)
                for h in range(HG):
                    q_head(h, tt, qlsb_t, ss_q, qcs)
                pending_u.append((tt, ss_k, ss_q, kuns, qcs))
            u_tail(*pending_u.pop(0))

        # ---- phase A: block-causal attention, query-block-major ----
        def a_tail(qb, den4, ycs):
            rinv = normf.tile([P, TQ], F32, name=f"rden_{qb}", tag="nf")
            nc.vector.reciprocal(rinv[:], den4[:])
            rbf = normb.tile([P, TQ], BF16, name=f"rdenb_{qb}", tag="nb")
            nc.vector.tensor_copy(out=rbf[:], in_=rinv[:])
            for h in range(HG):
                bc = ps_tile(f"abc_{h}_{qb}")
                nc.tensor.matmul(bc[:], sels[h][:], rbf[:], start=True, stop=True)
                nc.vector.tensor_mul(yTn_sb[:, h, qb * TQ:(qb + 1) * TQ],
                                     ycs[h][:], bc[:])

        pending_a = []
        for qb in range(NQB):
            # memset to 1.0 (not 0): unused rows go through reciprocal and 1/0=inf
            # would poison the selector matmul with 0*inf=NaN
            den4 = ps_tile(f"den4_{qb}")
            nc.vector.memset(den4[:], 1.0)
            ycs = []
            nkt = 4 * (qb + 1)
            for h in range(HG):
                yt_ps = ps_tile(f"yt_ps_{h}_{qb}")
                acc = accpool.tile([P, TQ], BF16, name=f"acc_{h}_{qb}", tag="acc")
                for kt in range(nkt):
                    sc_ps = ps_tile(f"sc_ps_{h}_{qb}_{kt}")
                    nc.tensor.matmul(sc_ps[:], kTn_sb[:, h, kt * P:(kt + 1) * P],
                                     qTn_sb[:, h, qb * TQ:(qb + 1) * TQ],
                                     start=True, stop=True)
                    ex = expool.tile([P, TQ], BF16, name=f"ex_{h}_{qb}_{kt}", tag="ex")
                    nc.scalar.activation(ex[:], sc_ps[:], AF.Exp,
                                         bias=zeros128[:], scale=1.0)
                    jrel = kt - 4 * qb
                    if jrel >= 0:
                        if jrel > 0:
                            nc.vector.memset(ex[:, 0:P * jrel], 0.0)
                        nc.vector.tensor_mul(ex[:, P * jrel:P * (jrel + 1)],
                                             ex[:, P * jrel:P * (jrel + 1)], tri_sb[:])
                    if kt == 0:
                        nc.vector.tensor_copy(out=acc[:], in_=ex[:])
                    else:
                        nc.vector.tensor_add(acc[:], acc[:], ex[:])
                    nc.tensor.matmul(yt_ps[:], v_sb[:, kt, h * D:(h + 1) * D], ex[:],
                                     start=(kt == 0), stop=(kt == nkt - 1))
                row_mm(den4, h, ones_red[:], acc[:])
                yc = castpool.tile([P, TQ], BF16, name=f"yc_{h}_{qb}", tag="cast")
                nc.scalar.copy(out=yc[:], in_=yt_ps[:])
                ycs.append(yc)
                if pending_a and h == 1:
                    a_tail(*pending_a.pop(0))
            pending_a.append((qb, den4, ycs))
        a_tail(*pending_a.pop(0))

        # ---- phase O: output projection (row-shard of wo), weights stationary ----
        for ct in range(C // P):
            wo_t = wop.tile([P, HG, P], BF16, name=f"wo_t{ct}", tag="wo_t")
            nc.sync.dma_start(out=wo_t[:],
                              in_=wo[:, ct * P:(ct + 1) * P].rearrange("(h p) c -> p h c", p=P))
            ops = [ps_tile(f"o_ps_{ct}_{tt}") for tt in range(NTT)]
            for h in range(HG):
                for tt in range(NTT):
                    nc.tensor.matmul(ops[tt][:], wo_t[:, h, :],
                                     yTn_sb[:, h, tt * TQ:(tt + 1) * TQ],
                                     start=(h == 0), stop=(h == HG - 1))
            for tt in range(NTT):
                o_sb = opool.tile([P, TQ], F32, name=f"o_sb_{ct}_{tt}", tag="o_sb")
                nc.vector.tensor_copy(out=o_sb[:], in_=ops[tt][:])
                nc.sync.dma_start(out=outT[ct * P:(ct + 1) * P, tt * TQ:(tt + 1) * TQ],
                                  in_=o_sb[:])

    nc.compile()
    return nc


def _get_nc(use_ag=USE_AG):
    if use_ag not in _NC_CACHE:
        _NC_CACHE[use_ag] = build_nc(use_ag)
    return _NC_CACHE[use_ag]


def _prepare_in_maps(x, cos, sin, wq_a, wq_b, wkv_a, wk_b, wkpe_b, wv_b, wo, use_ag=USE_AG):
    def bf(a):
        return np.ascontiguousarray(a).astype(NPBF16)

    cosT = bf(np.asarray(cos, np.float32)[0, :, 0, :].T)
    sinT = bf(np.asarray(sin, np.float32)[0, :, 0, :].T)
    tri = (np.arange(P)[:, None] <= np.arange(P)[None, :]).astype(NPBF16)

    wq_a_b, wkv_a_b = bf(wq_a), bf(wkv_a)
    wq_b_b, wk_b_b = bf(wq_b), bf(wk_b)
    wkpe_b_b, wv_b_b, wo_b = bf(wkpe_b), bf(wv_b), bf(wo)
    x = np.asarray(x, np.float32)

    in_maps = []
    for c in range(8):
        b, r = c // 4, c % 4
        if use_ag:
            xT_c = bf(x[b, r * TQ:(r + 1) * TQ, :].T)
        else:
            xT_c = bf(x[b].T)
        hgs = slice(r * HG * D, (r + 1) * HG * D)
        in_maps.append({
            "xT": xT_c,
            "wq_a": wq_a_b,
            "wkv_a": wkv_a_b,
            "wq_b": np.ascontiguousarray(wq_b_b[:, hgs]),
            "wk_b": np.ascontiguousarray(wk_b_b[:, hgs]),
            "wkpe_b": np.ascontiguousarray(wkpe_b_b[:, hgs]),
            "wv_b": np.ascontiguousarray(wv_b_b[:, hgs]),
            "wo": np.ascontiguousarray(wo_b[hgs, :]),
            "cosT": cosT,
            "sinT": sinT,
            "tri": tri,
        })
    return in_maps


def _assemble(results):
    out = np.empty((B, T, C), np.float32)
    for b in range(B):
        acc = results[4 * b]["outT"].astype(np.float32).copy()
        for r in range(1, 4):
            acc += results[4 * b + r]["outT"]
        out[b] = acc.T
    return out


def _run(inputs, use_ag=USE_AG, trace=False):
    nc = _get_nc(use_ag)
    in_maps = _prepare_in_maps(use_ag=use_ag, **inputs)
    res = run_bass_kernel_spmd(nc, in_maps, core_ids=list(range(8)), trace=trace)
    return _assemble(res.results), res


def kernel(**inputs):
    out, _ = _run(inputs)
    return out
